# revision 9
# baseline (speedup 1.0000x reference)
"""Trainium2 Bass kernel for nn_Attention_7078106104284.

Self-attention block (SAGAN-style) over x[8, 256, 64, 64]:
  q = wq@x+bq [32,n], k = wk@x+bk [32,n], v = wv@x+bv [256,n], n = 4096
  attn = softmax(q^T k, axis=m);  y = x + gamma * (v @ attn^T)

Sharding: data-parallel over batch — one batch element per NeuronCore (8 cores).

Per-core algorithm (matmuls in float32r — fp32 data, single-pass PE mode):
  - Projections: q,k as [32, 4096] (o on partitions), then replicated to all
    four 32-partition quadrants so the logit matmuls can be row-tiled; v is
    computed directly TRANSPOSED as vT [4096, 256] (m on partitions) using x
    chunks as the stationary operand, so attention needs no transposes.
  - Logits computed TRANSPOSED: Lt[m, n] = sum_o k[o,m] q[o,n] per n-group
    of 512 columns; 4 m-chunks per round via tile_position row-tiling (K=32
    uses a quarter of the PE array; 4 concurrent matmuls fill it).
  - exp fused with PSUM->SBUF evacuation on ACT, [128, 2048] per instruction.
    Softmax max-subtraction is skipped: logits ~ N(0, 32), |logit| < 50 << 88.
  - Z[n] = sum_m e[m,n]: two levels of pairwise adds on DVE (bf16 out, exact
    enough for a positive sum) + one ones-column bf16 matmul per round.
  - u[c, n] = sum_m vT[m,c] e[m,n] accumulated over 32 chunks in PSUM.
  - Epilogue: y = x + gamma*bv + u * broadcast(gamma/Z); the partition
    broadcast of gamma/Z is one ones-row matmul; v's bias folds into the
    residual because sum_m attn = 1.
"""

import sys

sys.path.insert(0, "/opt/trn_rl_repo")

import numpy as np
from contextlib import ExitStack

import concourse.bass as bass
import concourse.bacc as bacc
import concourse.tile as tile
import concourse.mybir as mybir
from concourse.bass_utils import run_bass_kernel_spmd

dt = mybir.dt
AF = mybir.ActivationFunctionType

B = 8
C = 256
C8 = 32
N = 4096          # h*w spatial positions
NG = 512          # n-group width (one PSUM bank of fp32)
G = N // NG       # 8 n-groups
MC = N // 128     # 32 m-chunks
RND = MC // 4     # 8 row-tiled rounds per group


def build_program(reps=1):
    nc = bacc.Bacc("TRN2", target_bir_lowering=False)
    f32 = dt.float32
    bf16 = dt.bfloat16
    r = dt.float32r
    x_d = nc.declare_dram_parameter("x", [C, N], r, isOutput=False)
    wqT_d = nc.declare_dram_parameter("wqT", [C, C8], r, isOutput=False)
    wkT_d = nc.declare_dram_parameter("wkT", [C, C8], r, isOutput=False)
    wvT_d = nc.declare_dram_parameter("wvT", [C, C], r, isOutput=False)
    bq_d = nc.declare_dram_parameter("bq", [C8, 1], f32, isOutput=False)
    bk_d = nc.declare_dram_parameter("bk", [C8, 1], f32, isOutput=False)
    bv_d = nc.declare_dram_parameter("bv", [128, 2], f32, isOutput=False)
    gamma_d = nc.declare_dram_parameter("gamma", [1, 1], f32, isOutput=False)
    y_d = nc.declare_dram_parameter("y", [C, N], f32, isOutput=True)

    with tile.TileContext(nc) as tc, ExitStack() as ctx:
        sing = ctx.enter_context(tc.tile_pool(name="sing", bufs=1))
        epool = ctx.enter_context(tc.tile_pool(name="epool", bufs=9))
        spool = ctx.enter_context(tc.tile_pool(name="spool", bufs=4))
        ypool = ctx.enter_context(tc.tile_pool(name="ypool", bufs=3))
        scal = ctx.enter_context(tc.tile_pool(name="scal", bufs=2))
        bcp = ctx.enter_context(tc.tile_pool(name="bcp", bufs=2))

        lt_ps = ctx.enter_context(tc.tile_pool(name="lt_ps", bufs=1, space="PSUM"))
        u_ps = ctx.enter_context(tc.tile_pool(name="u_ps", bufs=1, space="PSUM"))
        zb_ps = ctx.enter_context(tc.tile_pool(name="zb_ps", bufs=1, space="PSUM"))

        for _rep in range(reps):
            # ---- static inputs ----
            x_sb = sing.tile([128, 2, N], r)            # x[ch, m], ch = cc*128+p
            nc.sync.dma_start(out=x_sb, in_=x_d[:].rearrange("(cc p) m -> p cc m", p=128))
            x_f32 = x_sb.bitcast(f32)                   # read-only fp32 view
            wqT_sb = sing.tile([128, 2, C8], r)
            nc.sync.dma_start(out=wqT_sb, in_=wqT_d[:].rearrange("(cc p) o -> p cc o", p=128))
            wkT_sb = sing.tile([128, 2, C8], r)
            nc.sync.dma_start(out=wkT_sb, in_=wkT_d[:].rearrange("(cc p) o -> p cc o", p=128))
            wvT_sb = sing.tile([128, 2, C], r)
            nc.sync.dma_start(out=wvT_sb, in_=wvT_d[:].rearrange("(cc p) c -> p cc c", p=128))
            bq_sb = sing.tile([C8, 1], f32)
            nc.sync.dma_start(out=bq_sb, in_=bq_d[:])
            bk_sb = sing.tile([C8, 1], f32)
            nc.sync.dma_start(out=bk_sb, in_=bk_d[:])
            bv_sb = sing.tile([128, 2], f32)
            nc.sync.dma_start(out=bv_sb, in_=bv_d[:])
            # gamma broadcast to all 128 partitions via 0-stride DMA
            g128 = sing.tile([128, 1], f32)
            nc.sync.dma_start(
                out=g128,
                in_=bass.AP(tensor=gamma_d, offset=0, ap=[[0, 128], [1, 1]]),
            )

            ones_f32 = sing.tile([128, 1], f32)
            nc.vector.memset(ones_f32, 1.0)
            ones_row_f32 = sing.tile([1, 128], f32)
            nc.vector.memset(ones_row_f32, 1.0)
            ones_bf = sing.tile([128, 1], bf16)         # Z reduction stationary
            nc.vector.memset(ones_bf, 1.0)
            ones_row = sing.tile([1, 128], r)           # partition broadcast lhsT
            nc.scalar.activation(ones_row, ones_row_f32, AF.Copy)

            # gbv[c] = gamma * bv[c]  (per-partition adder for the epilogue)
            gbv = sing.tile([128, 2], f32)
            nc.vector.tensor_scalar_mul(gbv, bv_sb, g128)

            # ---- projections ----
            # q/k land in partitions 0-31, then are replicated to the other
            # three 32-partition quadrants for row-tiled logit matmuls.
            q_rep = sing.tile([128, N], r)
            k_rep = sing.tile([128, N], r)
            for s in range(G):
                sl = slice(s * NG, (s + 1) * NG)
                pq = lt_ps.tile([C8, NG], f32, tag="plt4", name="pq")
                pk = lt_ps.tile([C8, NG], f32, tag="plt4", name="pk")
                for cc in range(2):
                    nc.tensor.matmul(pq, wqT_sb[:, cc, :], x_sb[:, cc, sl],
                                     start=(cc == 0), stop=(cc == 1))
                for cc in range(2):
                    nc.tensor.matmul(pk, wkT_sb[:, cc, :], x_sb[:, cc, sl],
                                     start=(cc == 0), stop=(cc == 1))
                nc.scalar.activation(q_rep[0:C8, sl], pq, AF.Identity, bias=bq_sb)
                nc.scalar.activation(k_rep[0:C8, sl], pk, AF.Identity, bias=bk_sb)
            for rg in range(1, 4):
                po = slice(rg * C8, (rg + 1) * C8)
                nc.sync.dma_start(out=q_rep[po, :], in_=q_rep[0:C8, :])
                nc.sync.dma_start(out=k_rep[po, :], in_=k_rep[0:C8, :])

            vt_sb = sing.tile([128, MC, C], r)          # vT[m, c] (no bias)
            for mc in range(MC):
                msl = slice(mc * 128, (mc + 1) * 128)
                pv = lt_ps.tile([128, C], f32, tag="plt4", name="pv")
                for cc in range(2):
                    nc.tensor.matmul(pv, x_sb[:, cc, msl], wvT_sb[:, cc, :],
                                     start=(cc == 0), stop=(cc == 1))
                nc.scalar.activation(vt_sb[:, mc, :], pv, AF.Copy)

            # ---- attention, software-pipelined over n-groups ----
            e_tiles = {}
            u_tiles = {}
            z_tiles = {}

            def issue_lt_exp(g, j):
                # round j: logits for m-chunks 4j..4j+3 via 4 row-tiled
                # K=32 matmuls (one per 32-partition quadrant), one PSUM
                # bank each, one fused exp over all four banks.
                sl = slice(g * NG, (g + 1) * NG)
                plt4 = lt_ps.tile([128, 4, NG], dt.float32, tag="plt4", name="plt4")
                for rg in range(4):
                    mc = 4 * j + rg
                    msl = slice(mc * 128, (mc + 1) * 128)
                    po = slice(rg * C8, (rg + 1) * C8)
                    nc.tensor.matmul(plt4[:, rg, :], k_rep[po, msl], q_rep[po, sl],
                                     start=True, stop=True,
                                     tile_position=(rg * C8, 0))
                e4 = epool.tile([128, 4, NG], dt.float32r, tag="e4", name="e4")
                nc.scalar.activation(e4, plt4, AF.Exp)
                e_tiles[(g, j)] = e4

            def issue_av(g, j):
                u0, u1 = u_tiles[g]
                z_t = z_tiles[g]
                e4 = e_tiles.pop((g, j))
                ef = e4.bitcast(dt.float32)
                for rg in range(4):
                    mc = 4 * j + rg
                    first = (j == 0 and rg == 0)
                    last = (j == RND - 1 and rg == 3)
                    nc.tensor.matmul(u0, vt_sb[:, mc, 0:128], e4[:, rg, :],
                                     start=first, stop=last)
                    nc.tensor.matmul(u1, vt_sb[:, mc, 128:256], e4[:, rg, :],
                                     start=first, stop=last)
                # Z partial: two levels of pairwise adds (DVE), then one
                # bf16 ones-matmul per round.
                s01 = spool.tile([128, NG], dt.bfloat16, tag="s01", name="s01")
                nc.vector.tensor_add(s01, ef[:, 0, :], ef[:, 1, :])
                s23 = spool.tile([128, NG], dt.bfloat16, tag="s23", name="s23")
                nc.vector.tensor_add(s23, ef[:, 2, :], ef[:, 3, :])
                s03 = spool.tile([128, NG], dt.bfloat16, tag="s03", name="s03")
                nc.vector.tensor_add(s03, s01, s23)
                nc.tensor.matmul(z_t, ones_bf, s03,
                                 start=(j == 0), stop=(j == RND - 1))

            def issue_epilogue(g):
                sl = slice(g * NG, (g + 1) * NG)
                u0, u1 = u_tiles.pop(g)
                z_t = z_tiles.pop(g)
                rinv = scal.tile([1, NG], dt.float32, tag="rinv", name="rinv")
                nc.vector.reciprocal(rinv, z_t)
                srow = scal.tile([1, NG], dt.float32r, tag="srow", name="srow")
                nc.vector.tensor_scalar_mul(srow, rinv, g128[0:1, :])
                bc = zb_ps.tile([128, NG], dt.float32, tag="bc", name="bc")
                nc.tensor.matmul(bc, ones_row, srow, start=True, stop=True)
                bc_sb = bcp.tile([128, NG], dt.float32, tag="bcs", name="bcs")
                nc.scalar.activation(bc_sb, bc, AF.Copy)
                for cb, u in ((0, u0), (1, u1)):
                    y_t = ypool.tile([128, NG], dt.float32, tag="y", name="y")
                    nc.vector.tensor_mul(y_t, u, bc_sb)
                    nc.vector.tensor_add(y_t, y_t, x_f32[:, cb, sl])
                    nc.vector.tensor_scalar_add(y_t, y_t, gbv[:, cb:cb + 1])
                    nc.sync.dma_start(
                        out=y_d[:].rearrange("(cc p) m -> p cc m", p=128)[:, cb, sl],
                        in_=y_t,
                    )

            for g in range(G + 1):
                if g < G:
                    u_tiles[g] = (u_ps.tile([128, NG], dt.float32, tag="u0", name="u0"),
                                  u_ps.tile([128, NG], dt.float32, tag="u1", name="u1"))
                    z_tiles[g] = zb_ps.tile([1, NG], dt.float32, tag="z", name="z")
                for j in range(RND):
                    if g < G:
                        issue_lt_exp(g, j)
                    if g >= 1:
                        issue_av(g - 1, j)
                if g >= 1:
                    issue_epilogue(g - 1)

    nc.compile()
    return nc


_nc_cache = None


def kernel(**inputs) -> np.ndarray:
    global _nc_cache
    x = np.asarray(inputs["x"], dtype=np.float32)
    wq = np.asarray(inputs["wq"], dtype=np.float32)
    bq = np.asarray(inputs["bq"], dtype=np.float32)
    wk = np.asarray(inputs["wk"], dtype=np.float32)
    bk = np.asarray(inputs["bk"], dtype=np.float32)
    wv = np.asarray(inputs["wv"], dtype=np.float32)
    bv = np.asarray(inputs["bv"], dtype=np.float32)
    gamma = np.asarray(inputs["gamma"], dtype=np.float32)

    if _nc_cache is None:
        _nc_cache = build_program()
    nc = _nc_cache

    xr = np.ascontiguousarray(x.reshape(B, C, N))
    shared = {
        "wqT": np.ascontiguousarray(wq.T),
        "wkT": np.ascontiguousarray(wk.T),
        "wvT": np.ascontiguousarray(wv.T),
        "bq": np.ascontiguousarray(bq.reshape(C8, 1)),
        "bk": np.ascontiguousarray(bk.reshape(C8, 1)),
        "bv": np.ascontiguousarray(bv.reshape(2, 128).T),
        "gamma": np.ascontiguousarray(gamma.reshape(1, 1)),
    }
    in_maps = [dict(shared, x=xr[i]) for i in range(B)]
    res = run_bass_kernel_spmd(nc, in_maps, core_ids=list(range(B)))
    y = np.stack([res.results[i]["y"] for i in range(B)], axis=0)
    return y.reshape(B, C, 64, 64).astype(np.float32)


if __name__ == "__main__":
    rng = np.random.default_rng(0)
    ins = {
        "x": rng.standard_normal((B, C, 64, 64), dtype=np.float32),
        "wq": rng.standard_normal((C8, C), dtype=np.float32) / 16,
        "bq": rng.standard_normal((C8,), dtype=np.float32) * 0.01,
        "wk": rng.standard_normal((C8, C), dtype=np.float32) / 16,
        "bk": rng.standard_normal((C8,), dtype=np.float32) * 0.01,
        "wv": rng.standard_normal((C, C), dtype=np.float32) / 16,
        "bv": rng.standard_normal((C,), dtype=np.float32) * 0.01,
        "gamma": rng.standard_normal((1,), dtype=np.float32) * 0.1,
    }
    out = kernel(**ins)
    print("kernel output", out.shape, out.dtype)


# revision 13
# speedup vs baseline: 17.7126x; 17.7126x over previous
"""Trainium2 Bass kernel for nn_Attention_7078106104284.

Self-attention block (SAGAN-style) over x[8, 256, 64, 64]:
  q = wq@x+bq [32,n], k = wk@x+bk [32,n], v = wv@x+bv [256,n], n = 4096
  attn = softmax(q^T k, axis=m);  y = x + gamma * (v @ attn^T)

Sharding: data-parallel over batch — one batch element per NeuronCore (8 cores).

Per-core algorithm:
  - Projections (float32r matmuls): q,k as [32, 4096] (o on partitions),
    replicated to partitions 32-63 for 2-wide row-tiled logit matmuls; v is
    computed directly TRANSPOSED as vT [4096, 257] in bf16, where column 256
    is a ones column (written by a tiny K=1 matmul into PSUM) that makes the
    attention row-sum Z ride along the AV product for free.
  - Logits TRANSPOSED (float32r): Lt[m, n] = sum_o k[o,m] q[o,n] per n-group
    of 512; two m-chunks per round via tile_position row-tiling.
  - exp fused with PSUM->SBUF evacuation on ACT, bf16 out. Max-subtraction
    is skipped: logits ~ N(0, 32), |logit| < 50 << 88 (fp32 exp range).
  - AV transposed (bf16, FWL-fast weight loads): uT[n, c'] = sum_m e[m, n]
    vT[m, c'] per 128-wide n-sub-block, c' = 0..256 where uT[:, 256] = Z.
  - Normalize in uT space on DVE: uTn = uT * (gamma/Z[n]) (per-partition
    scalar), bf16 out; PE transposes uTn back to [c, n] (identity matmul).
  - Epilogue: y = trans + x + gamma*bv (v's bias folds into the residual
    because sum_m attn = 1).
"""

import sys

sys.path.insert(0, "/opt/trn_rl_repo")

import numpy as np
from contextlib import ExitStack

import concourse.bass as bass
import concourse.bacc as bacc
import concourse.tile as tile
import concourse.mybir as mybir
from concourse.masks import make_identity
from concourse.bass_utils import run_bass_kernel_spmd

dt = mybir.dt
AF = mybir.ActivationFunctionType

B = 8
C = 256
C8 = 32
N = 4096          # h*w spatial positions
NG = 512          # n-group width (one PSUM bank of fp32)
G = N // NG       # 8 n-groups
MC = N // 128     # 32 m-chunks
RW = 2            # row-tile width (concurrent K=32 logit matmuls)
RND = MC // RW    # rounds per group
CP = C + 1        # AV output channels incl. the Z ones-column


def build_program(reps=1):
    nc = bacc.Bacc("TRN2", target_bir_lowering=False)
    f32 = dt.float32
    bf16 = dt.bfloat16
    r = dt.float32r
    x_d = nc.declare_dram_parameter("x", [C, N], r, isOutput=False)
    wqT_d = nc.declare_dram_parameter("wqT", [C, C8], r, isOutput=False)
    wkT_d = nc.declare_dram_parameter("wkT", [C, C8], r, isOutput=False)
    wvT_d = nc.declare_dram_parameter("wvT", [C, C], r, isOutput=False)
    bq_d = nc.declare_dram_parameter("bq", [C8, 1], f32, isOutput=False)
    bk_d = nc.declare_dram_parameter("bk", [C8, 1], f32, isOutput=False)
    bv_d = nc.declare_dram_parameter("bv", [128, 2], f32, isOutput=False)
    gamma_d = nc.declare_dram_parameter("gamma", [1, 1], f32, isOutput=False)
    y_d = nc.declare_dram_parameter("y", [C, N], f32, isOutput=True)

    with tile.TileContext(nc) as tc, ExitStack() as ctx:
        sing = ctx.enter_context(tc.tile_pool(name="sing", bufs=1))
        epool = ctx.enter_context(tc.tile_pool(name="epool", bufs=2 * RND + 2))
        upool = ctx.enter_context(tc.tile_pool(name="upool", bufs=6))
        ypool = ctx.enter_context(tc.tile_pool(name="ypool", bufs=3))
        scal = ctx.enter_context(tc.tile_pool(name="scal", bufs=4))

        lt_ps = ctx.enter_context(tc.tile_pool(name="lt_ps", bufs=1, space="PSUM"))
        u_ps = ctx.enter_context(tc.tile_pool(name="u_ps", bufs=1, space="PSUM"))
        t_ps = ctx.enter_context(tc.tile_pool(name="t_ps", bufs=2, space="PSUM"))

        for _rep in range(reps):
            # ---- static inputs ----
            x_sb = sing.tile([128, 2, N], r)            # x[ch, m], ch = cc*128+p
            nc.sync.dma_start(out=x_sb, in_=x_d[:].rearrange("(cc p) m -> p cc m", p=128))
            x_f32 = x_sb.bitcast(f32)                   # read-only fp32 view
            wqT_sb = sing.tile([128, 2, C8], r)
            nc.sync.dma_start(out=wqT_sb, in_=wqT_d[:].rearrange("(cc p) o -> p cc o", p=128))
            wkT_sb = sing.tile([128, 2, C8], r)
            nc.sync.dma_start(out=wkT_sb, in_=wkT_d[:].rearrange("(cc p) o -> p cc o", p=128))
            wvT_sb = sing.tile([128, 2, C], r)
            nc.sync.dma_start(out=wvT_sb, in_=wvT_d[:].rearrange("(cc p) c -> p cc c", p=128))
            bq_sb = sing.tile([C8, 1], f32)
            nc.sync.dma_start(out=bq_sb, in_=bq_d[:])
            bk_sb = sing.tile([C8, 1], f32)
            nc.sync.dma_start(out=bk_sb, in_=bk_d[:])
            bv_sb = sing.tile([128, 2], f32)
            nc.sync.dma_start(out=bv_sb, in_=bv_d[:])
            # gamma broadcast to all 128 partitions via 0-stride DMA
            g128 = sing.tile([128, 1], f32)
            nc.sync.dma_start(
                out=g128,
                in_=bass.AP(tensor=gamma_d, offset=0, ap=[[0, 128], [1, 1]]),
            )

            ones_f32 = sing.tile([128, 1], f32)
            nc.vector.memset(ones_f32, 1.0)
            one_b = sing.tile([1, 1], bf16)             # K=1 ones-column writer
            nc.scalar.activation(one_b, ones_f32[0:1, :], AF.Copy)
            one_row_b = sing.tile([1, 128], bf16)
            nc.scalar.activation(
                one_row_b, bass.AP(tensor=ones_f32.tensor, offset=ones_f32.offset,
                                   ap=[[1, 1], [0, 128]]), AF.Copy)
            ident = sing.tile([128, 128], bf16)         # transpose identity
            make_identity(nc, ident)

            # gbv[c] = gamma * bv[c]  (per-partition adder for the epilogue)
            gbv = sing.tile([128, 2], f32)
            nc.vector.tensor_scalar_mul(gbv, bv_sb, g128)

            # ---- projections ----
            # q/k land in partitions 0-31, replicated once to 32-63.
            q_rep = sing.tile([2 * C8, N], r)
            k_rep = sing.tile([2 * C8, N], r)
            for s in range(G):
                sl = slice(s * NG, (s + 1) * NG)
                pq = lt_ps.tile([C8, NG], f32, tag="plt", name="pq")
                pk = lt_ps.tile([C8, NG], f32, tag="plt", name="pk")
                for cc in range(2):
                    nc.tensor.matmul(pq, wqT_sb[:, cc, :], x_sb[:, cc, sl],
                                     start=(cc == 0), stop=(cc == 1))
                for cc in range(2):
                    nc.tensor.matmul(pk, wkT_sb[:, cc, :], x_sb[:, cc, sl],
                                     start=(cc == 0), stop=(cc == 1))
                nc.scalar.activation(q_rep[0:C8, sl], pq, AF.Identity, bias=bq_sb)
                nc.scalar.activation(k_rep[0:C8, sl], pk, AF.Identity, bias=bk_sb)
            nc.sync.dma_start(out=q_rep[C8:2 * C8, :], in_=q_rep[0:C8, :])
            nc.sync.dma_start(out=k_rep[C8:2 * C8, :], in_=k_rep[0:C8, :])

            # vT[m, c'] in bf16 with ones column at c' = 256
            vt_sb = sing.tile([128, MC, CP], bf16)
            for mc in range(MC):
                msl = slice(mc * 128, (mc + 1) * 128)
                pv = lt_ps.tile([128, CP], f32, tag="plt", name="pv")
                for cc in range(2):
                    nc.tensor.matmul(pv[:, 0:C], x_sb[:, cc, msl], wvT_sb[:, cc, :],
                                     start=(cc == 0), stop=(cc == 1))
                nc.tensor.matmul(pv[:, C:CP], one_row_b, one_b,
                                 start=True, stop=True)
                nc.scalar.activation(vt_sb[:, mc, :], pv, AF.Copy)

            # ---- attention, software-pipelined over n-groups ----
            e_tiles = {}
            u_tiles = {}

            def issue_lt_exp(g, j):
                # round j: logits for m-chunks RW*j..RW*j+RW-1 via row-tiled
                # K=32 matmuls, one PSUM bank each, one fused exp over both.
                sl = slice(g * NG, (g + 1) * NG)
                plt = lt_ps.tile([128, RW, NG], f32, tag="plt", name="plt")
                for rg in range(RW):
                    mc = RW * j + rg
                    msl = slice(mc * 128, (mc + 1) * 128)
                    po = slice(rg * C8, (rg + 1) * C8)
                    nc.tensor.matmul(plt[:, rg, :], k_rep[po, msl], q_rep[po, sl],
                                     start=True, stop=True,
                                     tile_position=(rg * C8, 0))
                e_t = epool.tile([128, RW, NG], bf16, tag="e", name="e_t")
                nc.scalar.activation(e_t, plt, AF.Exp)
                e_tiles[(g, j)] = e_t

            def issue_av(g, j):
                uts = u_tiles[g]
                e_t = e_tiles.pop((g, j))
                for rg in range(RW):
                    mc = RW * j + rg
                    first = (j == 0 and rg == 0)
                    last = (j == RND - 1 and rg == RW - 1)
                    for sub in range(4):
                        nc.tensor.matmul(uts[sub],
                                         e_t[:, rg, sub * 128:(sub + 1) * 128],
                                         vt_sb[:, mc, :],
                                         start=first, stop=last)

            def issue_epilogue(g):
                uts = u_tiles.pop(g)
                # normalize per n-sub-block, transpose back to [c, n]
                tps = [t_ps.tile([128, NG], bf16, tag="tp", name="tp") for _ in range(2)]
                for sub in range(4):
                    ut = uts[sub]
                    rinv = scal.tile([128, 1], f32, tag="rinv", name="rinv")
                    nc.vector.reciprocal(rinv, ut[:, C:CP])
                    gsc = scal.tile([128, 1], f32, tag="gsc", name="gsc")
                    nc.vector.tensor_scalar_mul(gsc, rinv, g128)
                    un = upool.tile([128, C], bf16, tag="un", name="un")
                    nc.vector.tensor_scalar_mul(un, ut[:, 0:C], gsc)
                    for cb in range(2):
                        nc.tensor.transpose(
                            tps[cb][:, sub * 128:(sub + 1) * 128],
                            un[:, cb * 128:(cb + 1) * 128], ident)
                sl = slice(g * NG, (g + 1) * NG)
                for cb in range(2):
                    y_t = ypool.tile([128, NG], f32, tag="y", name="y")
                    nc.vector.tensor_add(y_t, tps[cb], x_f32[:, cb, sl])
                    nc.vector.tensor_scalar_add(y_t, y_t, gbv[:, cb:cb + 1])
                    nc.sync.dma_start(
                        out=y_d[:].rearrange("(cc p) m -> p cc m", p=128)[:, cb, sl],
                        in_=y_t,
                    )

            for g in range(G + 1):
                if g < G:
                    u_tiles[g] = [u_ps.tile([128, CP], f32, tag=f"u{s}", name=f"u{s}")
                                  for s in range(4)]
                for j in range(RND):
                    if g < G:
                        issue_lt_exp(g, j)
                    if g >= 1:
                        issue_av(g - 1, j)
                if g >= 1:
                    issue_epilogue(g - 1)

    nc.compile()
    return nc


_nc_cache = None


def kernel(**inputs) -> np.ndarray:
    global _nc_cache
    x = np.asarray(inputs["x"], dtype=np.float32)
    wq = np.asarray(inputs["wq"], dtype=np.float32)
    bq = np.asarray(inputs["bq"], dtype=np.float32)
    wk = np.asarray(inputs["wk"], dtype=np.float32)
    bk = np.asarray(inputs["bk"], dtype=np.float32)
    wv = np.asarray(inputs["wv"], dtype=np.float32)
    bv = np.asarray(inputs["bv"], dtype=np.float32)
    gamma = np.asarray(inputs["gamma"], dtype=np.float32)

    if _nc_cache is None:
        _nc_cache = build_program()
    nc = _nc_cache

    xr = np.ascontiguousarray(x.reshape(B, C, N))
    shared = {
        "wqT": np.ascontiguousarray(wq.T),
        "wkT": np.ascontiguousarray(wk.T),
        "wvT": np.ascontiguousarray(wv.T),
        "bq": np.ascontiguousarray(bq.reshape(C8, 1)),
        "bk": np.ascontiguousarray(bk.reshape(C8, 1)),
        "bv": np.ascontiguousarray(bv.reshape(2, 128).T),
        "gamma": np.ascontiguousarray(gamma.reshape(1, 1)),
    }
    in_maps = [dict(shared, x=xr[i]) for i in range(B)]
    res = run_bass_kernel_spmd(nc, in_maps, core_ids=list(range(B)))
    y = np.stack([res.results[i]["y"] for i in range(B)], axis=0)
    return y.reshape(B, C, 64, 64).astype(np.float32)


if __name__ == "__main__":
    rng = np.random.default_rng(0)
    ins = {
        "x": rng.standard_normal((B, C, 64, 64), dtype=np.float32),
        "wq": rng.standard_normal((C8, C), dtype=np.float32) / 16,
        "bq": rng.standard_normal((C8,), dtype=np.float32) * 0.01,
        "wk": rng.standard_normal((C8, C), dtype=np.float32) / 16,
        "bk": rng.standard_normal((C8,), dtype=np.float32) * 0.01,
        "wv": rng.standard_normal((C, C), dtype=np.float32) / 16,
        "bv": rng.standard_normal((C,), dtype=np.float32) * 0.01,
        "gamma": rng.standard_normal((1,), dtype=np.float32) * 0.1,
    }
    out = kernel(**ins)
    print("kernel output", out.shape, out.dtype)
